# revision 4
# baseline (speedup 1.0000x reference)
"""Trainium2 Bass kernel for nn_Cross_Domain_Class_Alignment.

Reference computation (per sample b):
    mask0[b] = argmin_k || feature_s2t[b,:,r,c] - centroid_target[k] ||^2
    mask1[b] = argmin_k || feature_target[b,:,r,c] - centroid_s2t[k] ||^2
    both nearest-upsampled from (65,129) to (512,1024), int32.

Sharding: data-parallel over batch B=8 across 8 NeuronCores (1 sample/core).
Centroids are replicated.

Per-core dataflow (per mask):
  - features [256, 8385] streamed in 3 column-slices x 2 channel chunks
  - dist matmuls: stationary = feature tile [128c, <=128px],
    moving = 2*centroid^T chunk [128c, 19] -> psum [px, 19] accumulated
    over the two channel chunks (argmax of 2*f.c - |c|^2 == argmin dist)
  - scalar_tensor_tensor: s = 2*dots - csq  (csq broadcast [128,19])
  - vector max8 + max_index per 128-pixel tile -> argmin index (uint32)
  - PE-transpose of the [128, 66] index matrix + DRAM bounce to reshape
    flat pixel order into [65, 129]
  - column nearest-upsample via segmented broadcast copies -> E [65, 1024]
  - row nearest-upsample via one-hot gather matmul G^T @ E -> [512, 1024]
  - convert f32->int32 on the scalar engine, DMA out
"""

import numpy as np

B, C, h, w = 8, 256, 65, 129
K = 19
H, W = 512, 1024
HW = h * w            # 8385
PX_TILE = 128
NT = (HW + PX_TILE - 1) // PX_TILE   # 66 pixel tiles (last has 65 px)
TILES_PER_GROUP = 22
GROUPS = [(g, min(g + TILES_PER_GROUP, NT)) for g in range(0, NT, TILES_PER_GROUP)]


def _col_segments():
    """Segments of the nearest-neighbor column map ci[c'] = c'*129 // 1024.

    Returns list of (src_start, n_src, rep, dst_start): output columns
    [dst_start : dst_start + n_src*rep] replicate source columns
    [src_start : src_start + n_src] each rep times.
    """
    ci = (np.arange(W) * w) // W
    reps = np.bincount(ci, minlength=w)
    segs = []
    i, dst = 0, 0
    while i < w:
        j = i
        while j < w and reps[j] == reps[i]:
            j += 1
        segs.append((i, j - i, int(reps[i]), dst))
        dst += (j - i) * int(reps[i])
        i = j
    assert dst == W
    return segs


def _row_onehot():
    """G[s, r'] = 1.0 iff floor(r'*65/512) == s; shape [65, 512] f32."""
    ri = (np.arange(H) * h) // H
    return (ri[None, :] == np.arange(h)[:, None]).astype(np.float32)


def build_module(num_devices=8):
    import concourse.bass as bass
    import concourse.tile as tile
    from concourse import bacc, mybir

    f32 = mybir.dt.float32
    i32 = mybir.dt.int32
    u32 = mybir.dt.uint32

    nc = bacc.Bacc(
        "TRN2",
        target_bir_lowering=False,
        debug=False,
        enable_asserts=False,
        num_devices=num_devices,
    )

    f_s2t = nc.dram_tensor("feature_s2t", [C, HW], f32, kind="ExternalInput")
    f_tgt = nc.dram_tensor("feature_target", [C, HW], f32, kind="ExternalInput")
    c_s2t = nc.dram_tensor("centroid_s2t", [K, C], f32, kind="ExternalInput")
    c_tgt = nc.dram_tensor("centroid_target", [K, C], f32, kind="ExternalInput")
    out0 = nc.dram_tensor("out0", [H, W], i32, kind="ExternalOutput")
    out1 = nc.dram_tensor("out1", [H, W], i32, kind="ExternalOutput")

    ident_dram = nc.inline_tensor(np.eye(128, dtype=np.float32), name="ident_const")
    g_dram = nc.inline_tensor(_row_onehot(), name="rowgather_const")

    col_segs = _col_segments()
    X = mybir.AxisListType.X

    with tile.TileContext(nc) as tc:
        from contextlib import ExitStack

        with ExitStack() as ctx:
            const_p = ctx.enter_context(tc.tile_pool(name="const", bufs=1))
            feat_p = ctx.enter_context(tc.tile_pool(name="feat", bufs=3))
            s_p = ctx.enter_context(tc.tile_pool(name="s", bufs=2))
            mx_p = ctx.enter_context(tc.tile_pool(name="mx", bufs=4))
            pt_p = ctx.enter_context(tc.tile_pool(name="pt", bufs=2))
            m_p = ctx.enter_context(tc.tile_pool(name="m", bufs=2))
            oi_p = ctx.enter_context(tc.tile_pool(name="oi", bufs=3))
            ps_dist = ctx.enter_context(tc.tile_pool(name="psd", bufs=3, space="PSUM"))
            ps_misc = ctx.enter_context(tc.tile_pool(name="psm", bufs=2, space="PSUM"))
            ps_out = ctx.enter_context(tc.tile_pool(name="pso", bufs=3, space="PSUM"))
            dram_p = ctx.enter_context(tc.tile_pool(name="dram", bufs=2, space="DRAM"))

            # ---- constants ----
            ident_sb = const_p.tile([128, 128], f32, tag="ident")
            nc.sync.dma_start(out=ident_sb[:], in_=ident_dram[:, :])
            g_sb = const_p.tile([h, H], f32, tag="gmat")
            nc.sync.dma_start(out=g_sb[:], in_=g_dram[:, :])
            ones_sb = const_p.tile([1, 128], f32, tag="ones")
            nc.vector.memset(ones_sb[:], 1.0)

            # ---- per-pair centroid prep: centT chunks (x2) and csq bcast ----
            def prep_pair(cent_dram, pidx):
                cent_sb = const_p.tile([K, C], f32, tag=f"cent{pidx}")
                nc.sync.dma_start(out=cent_sb[:], in_=cent_dram[:, :])
                sq = const_p.tile([K, C], f32, tag=f"centsq{pidx}")
                nc.vector.tensor_mul(sq[:], cent_sb[:], cent_sb[:])
                csq = const_p.tile([K, 1], f32, tag=f"csq{pidx}")
                nc.vector.reduce_sum(csq[:], sq[:], axis=X)
                # csq [19,1] -> [1,19] via PE transpose
                pcs = ps_misc.tile([1, K], f32, tag="misc")
                nc.tensor.transpose(pcs[:], csq[:], ident_sb[:K, :K])
                csqT = const_p.tile([1, K], f32, tag=f"csqT{pidx}")
                nc.vector.tensor_copy(out=csqT[:], in_=pcs[:])
                # broadcast to 128 partitions: ones[1,128].T @ csqT[1,19]
                pb = ps_misc.tile([128, K], f32, tag="misc")
                nc.tensor.matmul(pb[:], ones_sb[:], csqT[:], start=True, stop=True)
                csqb = const_p.tile([128, K], f32, tag=f"csqb{pidx}")
                nc.vector.tensor_copy(out=csqb[:], in_=pb[:])
                # centT chunks, scaled by 2
                centT = []
                for cc in range(2):
                    pt = ps_misc.tile([128, K], f32, tag="misc")
                    nc.tensor.transpose(
                        pt[:], cent_sb[:, cc * 128 : (cc + 1) * 128], ident_sb[:K, :K]
                    )
                    ct = const_p.tile([128, K], f32, tag=f"centT{pidx}_{cc}")
                    nc.scalar.mul(ct[:], pt[:], 2.0)
                    centT.append(ct)
                return centT, csqb

            centT_tgt, csqb_tgt = prep_pair(c_tgt, 0)   # for mask0 (feature_s2t)
            centT_s2t, csqb_s2t = prep_pair(c_s2t, 1)   # for mask1 (feature_target)

            # ---- per-mask pipeline ----
            def process_mask(feat, centT, csqb, out_dram, midx):
                pt8 = pt_p.tile([128, NT * 8], u32, tag="pt8")
                for g0, g1 in GROUPS:
                    ntg = g1 - g0
                    px0 = g0 * PX_TILE
                    pxn = min(HW, g1 * PX_TILE) - px0
                    fg = []
                    for cc in range(2):
                        ft = feat_p.tile([128, pxn], f32, tag=f"feat{cc}")
                        nc.sync.dma_start(
                            out=ft[:, :pxn],
                            in_=feat[cc * 128 : (cc + 1) * 128, px0 : px0 + pxn],
                        )
                        fg.append(ft)
                    psd = ps_dist.tile([128, ntg * K], f32, tag="dist")
                    for j in range(ntg):
                        lpx = j * PX_TILE
                        wj = min(PX_TILE, pxn - lpx)
                        if wj < PX_TILE:
                            # partial pixel tile: zero the tail rows (32-aligned
                            # partition base) before the matmuls overwrite
                            # [0:wj], so the batched read sees initialized PSUM
                            base = (wj // 32) * 32
                            nc.vector.memset(psd[base:, j * K : (j + 1) * K], 0.0)
                        nc.tensor.matmul(
                            psd[:wj, j * K : (j + 1) * K],
                            fg[0][:, lpx : lpx + wj],
                            centT[0][:],
                            start=True,
                            stop=False,
                        )
                        nc.tensor.matmul(
                            psd[:wj, j * K : (j + 1) * K],
                            fg[1][:, lpx : lpx + wj],
                            centT[1][:],
                            start=False,
                            stop=True,
                        )
                    # s = 2*dots - csq   (csq broadcast across tiles)
                    sg = s_p.tile([128, ntg * K], f32, tag="s")
                    nc.vector.scalar_tensor_tensor(
                        out=sg[:].rearrange("p (t k) -> p t k", k=K),
                        in0=psd[:].rearrange("p (t k) -> p t k", k=K),
                        scalar=1.0,
                        in1=csqb[:].unsqueeze(1).broadcast_to([128, ntg, K]),
                        op0=mybir.AluOpType.mult,
                        op1=mybir.AluOpType.subtract,
                    )
                    for j in range(ntg):
                        t = g0 + j
                        mx = mx_p.tile([128, 8], f32, tag="mx")
                        nc.vector.max(mx[:], sg[:, j * K : (j + 1) * K])
                        nc.vector.max_index(
                            pt8[:, t * 8 : (t + 1) * 8],
                            mx[:],
                            sg[:, j * K : (j + 1) * K],
                        )
                # extract index column 0 per tile, as f32
                ptf = pt_p.tile([128, NT], f32, tag="ptf")
                nc.vector.tensor_copy(
                    out=ptf[:], in_=pt8[:].rearrange("p (t e) -> p t e", e=8)[:, :, 0]
                )
                # transpose to flat pixel order and bounce through DRAM
                ptt = ps_misc.tile([NT, 128], f32, tag="misc")
                nc.tensor.transpose(ptt[:], ptf[:], ident_sb[:, :])
                pttsb = pt_p.tile([NT, 128], f32, tag="pttsb")
                nc.vector.tensor_copy(out=pttsb[:], in_=ptt[:])
                scratch = dram_p.tile([NT, 128], f32, tag="scratch")
                nc.sync.dma_start(out=scratch[:], in_=pttsb[:])
                msb = m_p.tile([h, w], f32, tag="m")
                nc.sync.dma_start(
                    out=msb[:],
                    in_=scratch[:].rearrange("a b -> (a b)")[0:HW].rearrange(
                        "(r c) -> r c", c=w
                    ),
                )
                # column nearest-upsample 129 -> 1024
                e_sb = m_p.tile([h, W], f32, tag="e")
                for src0, nsrc, rep, dst0 in col_segs:
                    nc.vector.tensor_copy(
                        out=e_sb[:, dst0 : dst0 + nsrc * rep].rearrange(
                            "p (s r) -> p s r", r=rep
                        ),
                        in_=msb[:, src0 : src0 + nsrc]
                        .unsqueeze(2)
                        .broadcast_to([h, nsrc, rep]),
                    )
                # row nearest-upsample 65 -> 512 via one-hot gather matmul
                for n in range(H // 128):
                    oint = oi_p.tile([128, W], i32, tag="oint")
                    for hh in range(W // 512):
                        po = ps_out.tile([128, 512], f32, tag="out")
                        nc.tensor.matmul(
                            po[:],
                            g_sb[:, n * 128 : (n + 1) * 128],
                            e_sb[:, hh * 512 : (hh + 1) * 512],
                            start=True,
                            stop=True,
                        )
                        nc.scalar.copy(oint[:, hh * 512 : (hh + 1) * 512], po[:])
                    nc.sync.dma_start(
                        out=out_dram[n * 128 : (n + 1) * 128, :], in_=oint[:]
                    )

            process_mask(f_s2t, centT_tgt, csqb_tgt, out0, 0)
            process_mask(f_tgt, centT_s2t, csqb_s2t, out1, 1)

    nc.compile()
    return nc


_cached_nc = None


def _get_nc():
    global _cached_nc
    if _cached_nc is None:
        _cached_nc = build_module()
    return _cached_nc


def make_in_maps(feature_s2t, feature_target, centroid_s2t, centroid_target):
    in_maps = []
    for b in range(B):
        in_maps.append(
            {
                "feature_s2t": np.ascontiguousarray(
                    feature_s2t[b], dtype=np.float32
                ).reshape(C, HW),
                "feature_target": np.ascontiguousarray(
                    feature_target[b], dtype=np.float32
                ).reshape(C, HW),
                "centroid_s2t": np.ascontiguousarray(centroid_s2t, dtype=np.float32),
                "centroid_target": np.ascontiguousarray(
                    centroid_target, dtype=np.float32
                ),
            }
        )
    return in_maps


def kernel(
    feature_s2t,
    feature_target,
    centroid_s2t,
    centroid_target,
    seg_s2t=None,
    seg_target=None,
    **_unused,
):
    from concourse.bass_utils import run_bass_kernel_spmd

    nc = _get_nc()
    in_maps = make_in_maps(
        np.asarray(feature_s2t),
        np.asarray(feature_target),
        np.asarray(centroid_s2t),
        np.asarray(centroid_target),
    )
    res = run_bass_kernel_spmd(nc, in_maps, core_ids=list(range(B)))
    results = res.results
    m0 = np.stack([results[b]["out0"] for b in range(B)]).astype(np.int32)
    m1 = np.stack([results[b]["out1"] for b in range(B)]).astype(np.int32)
    return (m0, m1)


# revision 8
# speedup vs baseline: 1.7918x; 1.7918x over previous
"""Trainium2 Bass kernel for nn_Cross_Domain_Class_Alignment.

Reference computation (per sample b):
    mask0[b] = argmin_k || feature_s2t[b,:,r,c] - centroid_target[k] ||^2
    mask1[b] = argmin_k || feature_target[b,:,r,c] - centroid_s2t[k] ||^2
    both nearest-upsampled from (65,129) to (512,1024), int32.

Sharding: data-parallel over batch B=8 across 8 NeuronCores (1 sample/core).
Centroids are replicated.

Per-core dataflow (per mask):
  - features [256, 8385] streamed in 2048-pixel slices x 2 channel chunks
  - dist matmuls, centroid-stationary: psum quad [128, 512] holds four
    512-pixel banks stacked at partition offsets {0,32,64,96} via
    tile_position col-tiling (the 4 matmuls run concurrently in separate
    PE column groups).  Stationary = centT [128c, 32] (19 real cols +
    13 zero cols so all 32 partitions get written), moving = feature
    [128c, 512].  Two chunk matmuls accumulate C=256.
  - scalar-engine copy fuses m = 2*dots - csq (per-partition bias) while
    moving the quad PSUM->SBUF
  - PE transposes of [128,128] slices flip pixels onto partitions:
    out[128px, 4 groups x 32] -> batched DVE argmax via
    reduce_max / is_ge / *(19-k) / reduce_max (first-index tie-break)
  - y = 19 - argmin flows through: PE transpose of the [128, 66] block
    matrix + DRAM bounce reshapes flat pixel order into [65, 129]
  - column nearest-upsample via segmented broadcast copies -> E [65,1024] bf16
  - row nearest-upsample via one-hot gather matmul G^T @ E (bf16) ->
    [512, 1024], converted to idx = 19 - y and int32 on the scalar engine
"""

import numpy as np

B, C, h, w = 8, 256, 65, 129
K = 19
H, W = 512, 1024
HW = h * w              # 8385
QUAD_PX = 2048          # four 512-px banks per psum quad
NFULL = HW // QUAD_PX   # 4 full quads
REM = HW - NFULL * QUAD_PX   # 193 remainder pixels
NT = (HW + 127) // 128  # 66 pixel blocks of 128 (for the block matrix)


def _col_segments():
    """Segments of the nearest-neighbor column map ci[c'] = c'*129 // 1024."""
    ci = (np.arange(W) * w) // W
    reps = np.bincount(ci, minlength=w)
    segs = []
    i, dst = 0, 0
    while i < w:
        j = i
        while j < w and reps[j] == reps[i]:
            j += 1
        segs.append((i, j - i, int(reps[i]), dst))
        dst += (j - i) * int(reps[i])
        i = j
    assert dst == W
    return segs


def _row_onehot():
    """G[s, r'] = 1.0 iff floor(r'*65/512) == s; shape [65, 512] bf16."""
    import ml_dtypes

    ri = (np.arange(H) * h) // H
    return (ri[None, :] == np.arange(h)[:, None]).astype(ml_dtypes.bfloat16)


def build_module(num_devices=8):
    import concourse.bass as bass
    import concourse.tile as tile
    from concourse import bacc, mybir

    f32 = mybir.dt.float32
    bf16 = mybir.dt.bfloat16
    i32 = mybir.dt.int32

    nc = bacc.Bacc(
        "TRN2",
        target_bir_lowering=False,
        debug=False,
        enable_asserts=False,
        num_devices=num_devices,
    )

    f_s2t = nc.dram_tensor("feature_s2t", [C, HW], f32, kind="ExternalInput")
    f_tgt = nc.dram_tensor("feature_target", [C, HW], f32, kind="ExternalInput")
    c_s2t = nc.dram_tensor("centroid_s2t", [K, C], f32, kind="ExternalInput")
    c_tgt = nc.dram_tensor("centroid_target", [K, C], f32, kind="ExternalInput")
    out0 = nc.dram_tensor("out0", [H, W], i32, kind="ExternalOutput")
    out1 = nc.dram_tensor("out1", [H, W], i32, kind="ExternalOutput")

    ident_dram = nc.inline_tensor(np.eye(128, dtype=np.float32), name="ident_const")
    g_dram = nc.inline_tensor(_row_onehot(), name="rowgather_const")
    wk_np = np.tile((K - np.arange(K)).astype(np.float32), (128, 1))
    wk_dram = nc.inline_tensor(wk_np, name="wk_const")
    # sel[k, 32j+k] = -1.0: replicates -csq over the four 32-partition groups
    sel_np = np.zeros((K, 128), dtype=np.float32)
    for j in range(4):
        sel_np[np.arange(K), 32 * j + np.arange(K)] = -1.0
    sel_dram = nc.inline_tensor(sel_np, name="sel_const")

    col_segs = _col_segments()
    X = mybir.AxisListType.X
    ALU = mybir.AluOpType
    AF = mybir.ActivationFunctionType

    with tile.TileContext(nc) as tc:
        from contextlib import ExitStack

        with ExitStack() as ctx:
            const_p = ctx.enter_context(tc.tile_pool(name="const", bufs=1))
            feat_p = ctx.enter_context(tc.tile_pool(name="feat", bufs=3))
            q_p = ctx.enter_context(tc.tile_pool(name="q", bufs=3))
            s_p = ctx.enter_context(tc.tile_pool(name="s", bufs=2))
            pt_p = ctx.enter_context(tc.tile_pool(name="pt", bufs=2))
            m_p = ctx.enter_context(tc.tile_pool(name="m", bufs=2))
            oi_p = ctx.enter_context(tc.tile_pool(name="oi", bufs=3))
            ps_dist = ctx.enter_context(tc.tile_pool(name="psd", bufs=3, space="PSUM"))
            ps_tr = ctx.enter_context(tc.tile_pool(name="pst", bufs=3, space="PSUM"))
            ps_out = ctx.enter_context(tc.tile_pool(name="pso", bufs=2, space="PSUM"))
            dram_p = ctx.enter_context(tc.tile_pool(name="dram", bufs=2, space="DRAM"))

            # ---- constants ----
            ident = const_p.tile([128, 128], f32, tag="ident")
            nc.sync.dma_start(out=ident[:], in_=ident_dram[:, :])
            g_sb = const_p.tile([h, H], bf16, tag="gmat")
            nc.sync.dma_start(out=g_sb[:], in_=g_dram[:, :])
            wk_sb = const_p.tile([128, K], f32, tag="wk")
            nc.sync.dma_start(out=wk_sb[:], in_=wk_dram[:, :])
            sel_sb = const_p.tile([K, 128], f32, tag="sel")
            nc.sync.dma_start(out=sel_sb[:], in_=sel_dram[:, :])

            # ---- per-pair centroid prep ----
            def prep_pair(cent_dram, pidx):
                cent_sb = const_p.tile([K, C], f32, tag=f"cent{pidx}")
                nc.sync.dma_start(out=cent_sb[:], in_=cent_dram[:, :])
                sq = const_p.tile([K, C], f32, tag=f"centsq{pidx}")
                nc.vector.tensor_mul(sq[:], cent_sb[:], cent_sb[:])
                csq = const_p.tile([K, 1], f32, tag=f"csq{pidx}")
                nc.vector.reduce_sum(csq[:], sq[:], axis=X)
                # -csq replicated at partition offsets {0,32,64,96}
                pb = ps_tr.tile([128, 1], f32, tag="tr")
                nc.tensor.matmul(pb[:], sel_sb[:], csq[:], start=True, stop=True)
                csqn4 = const_p.tile([128, 1], f32, tag=f"csqn4_{pidx}")
                nc.vector.tensor_copy(out=csqn4[:], in_=pb[:])
                # centT chunks [128, 32]: cols 0:19 = cent^T, cols 19:32 = 0
                centT = []
                for cc in range(2):
                    ct = const_p.tile([128, 32], f32, tag=f"centT{pidx}_{cc}")
                    nc.vector.memset(ct[:], 0.0)
                    pt = ps_tr.tile([128, K], f32, tag="tr")
                    nc.tensor.transpose(
                        pt[:], cent_sb[:, cc * 128 : (cc + 1) * 128], ident[:K, :K]
                    )
                    nc.vector.tensor_copy(out=ct[:, 0:K], in_=pt[:])
                    centT.append(ct)
                return centT, csqn4

            centT_tgt, csqn4_tgt = prep_pair(c_tgt, 0)   # for mask0 (feature_s2t)
            centT_s2t, csqn4_s2t = prep_pair(c_s2t, 1)   # for mask1 (feature_target)

            # ---- per-mask pipeline ----
            def process_mask(feat, centT, csqn4, out_dram, midx):
                # sg layout: value for pixel block b (= p//128), class k at
                # column 19*b + k  (66 blocks x 19 = 1254, padded)
                sg = s_p.tile([128, NT * K + 40], f32, tag="s")
                for Bq in range(NFULL + 1):
                    px0 = Bq * QUAD_PX
                    pxw = min(QUAD_PX, HW - px0)
                    fg = []
                    for cc in range(2):
                        ft = feat_p.tile([128, QUAD_PX], f32, tag=f"feat{cc}")
                        nc.sync.dma_start(
                            out=ft[:, :pxw],
                            in_=feat[cc * 128 : (cc + 1) * 128, px0 : px0 + pxw],
                        )
                        fg.append(ft)
                    if Bq < NFULL:
                        # full quad: 4 col-groups x 2 chunks
                        psq = ps_dist.tile([128, 512], f32, tag="dist")
                        for j in range(4):
                            for cc in range(2):
                                nc.tensor.matmul(
                                    psq[32 * j : 32 * j + 32, :],
                                    centT[cc][:],
                                    fg[cc][:, 512 * j : 512 * j + 512],
                                    start=(cc == 0),
                                    stop=(cc == 1),
                                    tile_position=(0, 32 * j),
                                )
                        quad = q_p.tile([128, 512], f32, tag="quad")
                        nc.scalar.activation(
                            out=quad[:],
                            in_=psq[:],
                            func=AF.Identity,
                            bias=csqn4[:],
                            scale=2.0,
                        )
                        for tq in range(4):
                            ptr = ps_tr.tile([128, 128], f32, tag="tr")
                            nc.tensor.transpose(
                                ptr[:], quad[:, 128 * tq : 128 * tq + 128], ident[:]
                            )
                            # ptr free col = 32*j + k'; block b = 16*Bq + 4*j + tq
                            base = K * (16 * Bq + tq)
                            nc.vector.tensor_copy(
                                out=sg[:, base : base + 4 * 4 * K]
                                .rearrange("p (j e) -> p j e", e=4 * K)[:, :, 0:K],
                                in_=ptr[:]
                                .rearrange("p (j e) -> p j e", e=32)[:, :, 0:K],
                            )
                    else:
                        # remainder: 193 px, single group
                        psr = ps_dist.tile([32, 256], f32, tag="dist")
                        nc.vector.memset(psr[:, pxw:256], 0.0)
                        for cc in range(2):
                            nc.tensor.matmul(
                                psr[0:32, 0:pxw],
                                centT[cc][:],
                                fg[cc][:, 0:pxw],
                                start=(cc == 0),
                                stop=(cc == 1),
                            )
                        st2 = q_p.tile([32, 256], f32, tag="st2")
                        nc.scalar.activation(
                            out=st2[:],
                            in_=psr[:],
                            func=AF.Identity,
                            bias=csqn4[0:32, :],
                            scale=2.0,
                        )
                        for tq in range(2):
                            b = 64 + tq
                            ptr = ps_tr.tile([128, 32], f32, tag="tr")
                            nc.tensor.transpose(
                                ptr[:], st2[:, 128 * tq : 128 * tq + 128], ident[:32, :32]
                            )
                            nc.vector.tensor_copy(
                                out=sg[:, K * b : K * b + K],
                                in_=ptr[:, 0:K],
                            )

                # ---- batched argmax over k (y = 19 - argmin) ----
                ptf = pt_p.tile([128, NT], f32, tag="ptf")
                eq = s_p.tile([128, NT * K], f32, tag="eq")
                s3d = sg[:, 0 : NT * K].rearrange("p (b k) -> p b k", k=K)
                mx = pt_p.tile([128, NT], f32, tag="mx")
                nc.vector.tensor_reduce(mx[:], s3d, axis=X, op=ALU.max)
                eq3d = eq[:].rearrange("p (b k) -> p b k", k=K)
                nc.vector.tensor_tensor(
                    out=eq3d,
                    in0=s3d,
                    in1=mx[:].unsqueeze(2).broadcast_to([128, NT, K]),
                    op=ALU.is_ge,
                )
                nc.vector.tensor_tensor(
                    out=eq3d,
                    in0=eq3d,
                    in1=wk_sb[:].unsqueeze(1).broadcast_to([128, NT, K]),
                    op=ALU.mult,
                )
                nc.vector.tensor_reduce(ptf[:], eq3d, axis=X, op=ALU.max)

                # ---- block matrix -> flat pixel order (transpose + bounce) ----
                ptt = ps_tr.tile([NT, 128], f32, tag="tr")
                nc.tensor.transpose(ptt[:], ptf[:], ident[:])
                pttsb = pt_p.tile([NT, 128], f32, tag="pttsb")
                nc.vector.tensor_copy(out=pttsb[:], in_=ptt[:])
                scratch = dram_p.tile([NT, 128], f32, tag="scratch")
                nc.sync.dma_start(out=scratch[:], in_=pttsb[:])
                msb = m_p.tile([h, w], f32, tag="m")
                nc.sync.dma_start(
                    out=msb[:],
                    in_=scratch[:].rearrange("a b -> (a b)")[0:HW].rearrange(
                        "(r c) -> r c", c=w
                    ),
                )
                # ---- column nearest-upsample 129 -> 1024 (bf16) ----
                e_sb = m_p.tile([h, W], bf16, tag="e")
                for src0, nsrc, rep, dst0 in col_segs:
                    nc.vector.tensor_copy(
                        out=e_sb[:, dst0 : dst0 + nsrc * rep].rearrange(
                            "p (s r) -> p s r", r=rep
                        ),
                        in_=msb[:, src0 : src0 + nsrc]
                        .unsqueeze(2)
                        .broadcast_to([h, nsrc, rep]),
                    )
                # ---- row nearest-upsample 65 -> 512 + int convert + store ----
                for n in range(H // 128):
                    oint = oi_p.tile([128, W], i32, tag="oint")
                    for hh in range(W // 512):
                        po = ps_out.tile([128, 512], f32, tag="out")
                        nc.tensor.matmul(
                            po[:],
                            g_sb[:, n * 128 : (n + 1) * 128],
                            e_sb[:, hh * 512 : (hh + 1) * 512],
                            start=True,
                            stop=True,
                        )
                        # idx = 19 - y, cast to int32
                        nc.scalar.activation(
                            out=oint[:, hh * 512 : (hh + 1) * 512],
                            in_=po[:],
                            func=AF.Copy,
                            bias=float(K),
                            scale=-1.0,
                        )
                    nc.sync.dma_start(
                        out=out_dram[n * 128 : (n + 1) * 128, :], in_=oint[:]
                    )

            process_mask(f_s2t, centT_tgt, csqn4_tgt, out0, 0)
            process_mask(f_tgt, centT_s2t, csqn4_s2t, out1, 1)

    nc.compile()
    return nc


_cached_nc = None


def _get_nc():
    global _cached_nc
    if _cached_nc is None:
        _cached_nc = build_module()
    return _cached_nc


def make_in_maps(feature_s2t, feature_target, centroid_s2t, centroid_target):
    in_maps = []
    for b in range(B):
        in_maps.append(
            {
                "feature_s2t": np.ascontiguousarray(
                    feature_s2t[b], dtype=np.float32
                ).reshape(C, HW),
                "feature_target": np.ascontiguousarray(
                    feature_target[b], dtype=np.float32
                ).reshape(C, HW),
                "centroid_s2t": np.ascontiguousarray(centroid_s2t, dtype=np.float32),
                "centroid_target": np.ascontiguousarray(
                    centroid_target, dtype=np.float32
                ),
            }
        )
    return in_maps


def kernel(
    feature_s2t,
    feature_target,
    centroid_s2t,
    centroid_target,
    seg_s2t=None,
    seg_target=None,
    **_unused,
):
    from concourse.bass_utils import run_bass_kernel_spmd

    nc = _get_nc()
    in_maps = make_in_maps(
        np.asarray(feature_s2t),
        np.asarray(feature_target),
        np.asarray(centroid_s2t),
        np.asarray(centroid_target),
    )
    res = run_bass_kernel_spmd(nc, in_maps, core_ids=list(range(B)))
    results = res.results
    m0 = np.stack([results[b]["out0"] for b in range(B)]).astype(np.int32)
    m1 = np.stack([results[b]["out1"] for b in range(B)]).astype(np.int32)
    return (m0, m1)
